# revision 2
# baseline (speedup 1.0000x reference)
"""NoisyTopkRouter on 8 trn2 NeuronCores.

Problem: x[4,8192,2048] f32, Wg/Wn[8,2048], bg/bn[8], noise[4,8192,8], k=2.
  gate = x@Wg.T + bg ; pre = x@Wn.T + bn
  nl = noise*softplus(pre) + gate
  top2 -> probs = softmax(top2 logits) scattered back; also top2 indices.

Sharding: data parallel over batch*seq (32768 tokens -> 4096/core).
Weights replicated.

Device strategy (memory-bound: 33.5MB of x per core dominates):
- Host pre-transposes each core's x shard to [D, T] and splits fp32 into
  fp16 hi + fp16 residual*2048 (same total bytes as fp32; lets the PE run
  full-rate fp16 matmuls instead of quarter-rate fp32; verified max logit
  error 1.4e-6 < plain fp32's 2.5e-6).
- PE: per 512 tokens, 32 matmuls (16 D-chunks x {xh vs [Wh|Wl], xl vs
  [0|Wh]}) accumulate logitsT'[32,512] in one PSUM bank. Rows 0:16 carry
  xh@[Wh], rows 16:32 carry the 1/2048-scaled cross terms.
- ACT fuses scale+bias while copying rows 16:32 out of PSUM; DVE adds.
- 4 PE transposes flip logitsT [16,512] to token-major [128,4,16] PSUM.
- softplus = Ln(Exp(z)+1) (single ACT table set natural_log_exp).
- top-2 via DVE max8/max_index (sorted, tie semantics match jax top_k).
- 2-way softmax: p1 = 1/(1+e^(v2-v1)), p2 = e^(v2-v1)*p1.
- scatter via iota==idx compares * p, summed.
"""

import numpy as np

import concourse.bacc as bacc
import concourse.bass as bass
import concourse.mybir as mybir
import concourse.tile as tile
from concourse.bass_utils import run_bass_kernel_spmd

AFT = mybir.ActivationFunctionType
ALU = mybir.AluOpType
DT = mybir.dt

B, S, D, E = 4, 8192, 2048, 8
N_TOK = B * S
N_CORES = 8
TC = N_TOK // N_CORES          # tokens per core
SCALE = 2048.0                 # residual prescale (keeps fp16 normal)
NCHUNK = D // 128              # 16 contraction chunks


def build_program(tc_tokens=TC, slab=1024, loop_n=1):
    """Build + compile the per-core Bass program.

    tc_tokens: tokens this core processes (multiple of slab)
    slab: tokens per DMA slab (multiple of 512)
    loop_n: >1 wraps the body in a For_i for device-side timing runs
    """
    assert slab % 512 == 0 and tc_tokens % slab == 0
    nhalf = slab // 512

    nc = bacc.Bacc("TRN2", target_bir_lowering=False, debug=False)

    xh_d = nc.dram_tensor("xh", [D, tc_tokens], DT.float16, kind="ExternalInput")
    xl_d = nc.dram_tensor("xl", [D, tc_tokens], DT.float16, kind="ExternalInput")
    whwl_d = nc.dram_tensor("whwl", [D, 48], DT.float16, kind="ExternalInput")
    zwh_d = nc.dram_tensor("zwh", [D, 48], DT.float16, kind="ExternalInput")
    bcat_d = nc.dram_tensor("bcat", [16, 1], DT.float32, kind="ExternalInput")
    ident_d = nc.dram_tensor("ident", [16, 16], DT.float32, kind="ExternalInput")
    noise_d = nc.dram_tensor("noise", [tc_tokens, E], DT.float32, kind="ExternalInput")
    rout_d = nc.dram_tensor("rout", [tc_tokens, E], DT.float32, kind="ExternalOutput")
    texp_d = nc.dram_tensor("texp", [tc_tokens, 2], DT.int32, kind="ExternalOutput")

    xh_r = xh_d[:, :].rearrange("(c p) t -> p c t", p=128)
    xl_r = xl_d[:, :].rearrange("(c p) t -> p c t", p=128)

    with tile.TileContext(nc) as tc:
        import contextlib
        with contextlib.ExitStack() as ctx:
            consts = ctx.enter_context(tc.tile_pool(name="consts", bufs=1))
            xpool = ctx.enter_context(tc.tile_pool(name="xh", bufs=2))
            xlpool = ctx.enter_context(tc.tile_pool(name="xl", bufs=2))
            npool = ctx.enter_context(tc.tile_pool(name="noise", bufs=2))
            lpool = ctx.enter_context(tc.tile_pool(name="logits", bufs=3))
            opool = ctx.enter_context(tc.tile_pool(name="outs", bufs=3))
            pmm = ctx.enter_context(tc.tile_pool(name="pmm", bufs=2, space="PSUM"))
            ptr = ctx.enter_context(tc.tile_pool(name="ptr", bufs=2, space="PSUM"))

            whwl_t = consts.tile([128, NCHUNK, 48], DT.float16)
            nc.sync.dma_start(whwl_t[:], whwl_d[:, :].rearrange("(c p) k -> p c k", p=128))
            zwh_t = consts.tile([128, NCHUNK, 48], DT.float16)
            nc.sync.dma_start(zwh_t[:], zwh_d[:, :].rearrange("(c p) k -> p c k", p=128))
            ident_t = consts.tile([16, 16], DT.float32)
            nc.sync.dma_start(ident_t[:], ident_d[:, :])
            bcat_t = consts.tile([16, 1], DT.float32)
            nc.sync.dma_start(bcat_t[:], bcat_d[:, :])
            iota_t = consts.tile([128, E], DT.uint32)
            nc.gpsimd.iota(iota_t[:], pattern=[[1, E]], base=0, channel_multiplier=0)

            def body(_iv=None):
                for g in range(tc_tokens // slab):
                    xh_t = xpool.tile([128, NCHUNK, slab], DT.float16, tag="xh")
                    nc.sync.dma_start(xh_t[:], xh_r[:, :, g * slab:(g + 1) * slab])
                    xl_t = xlpool.tile([128, NCHUNK, slab], DT.float16, tag="xl")
                    nc.sync.dma_start(xl_t[:], xl_r[:, :, g * slab:(g + 1) * slab])
                    noise_t = npool.tile([128, slab // 128, E], DT.float32, tag="nz")
                    nc.sync.dma_start(
                        noise_t[:],
                        noise_d[g * slab:(g + 1) * slab, :].rearrange(
                            "(s p) e -> p s e", p=128))

                    for h in range(nhalf):
                        tok0 = g * slab + h * 512
                        psum1 = pmm.tile([48, 512], DT.float32, tag="mm")
                        for c in range(NCHUNK):
                            nc.tensor.matmul(
                                psum1[:], whwl_t[:, c, :],
                                xh_t[:, c, h * 512:(h + 1) * 512],
                                start=(c == 0), stop=False)
                        for c in range(NCHUNK):
                            nc.tensor.matmul(
                                psum1[:], zwh_t[:, c, :],
                                xl_t[:, c, h * 512:(h + 1) * 512],
                                start=False, stop=(c == NCHUNK - 1))

                        # cross terms * 1/SCALE + bias, PSUM -> SBUF
                        t_sb = lpool.tile([16, 512], DT.float32, tag="tsb")
                        nc.scalar.activation(t_sb[:], psum1[32:48, :], AFT.Identity,
                                             bias=bcat_t[:, 0:1], scale=1.0 / SCALE)
                        l_sb = lpool.tile([16, 512], DT.float32, tag="lsb")
                        nc.vector.tensor_add(l_sb[:], psum1[0:16, :], t_sb[:])

                        # transpose to token-major [128, 4, 16]
                        psum_t = ptr.tile([128, 4, 16], DT.float32, tag="tr")
                        for s in range(4):
                            nc.tensor.transpose(psum_t[:, s, :],
                                                l_sb[:, s * 128:(s + 1) * 128],
                                                ident_t[:])
                        gate_ap = psum_t[:, :, 0:8]
                        pre_ap = psum_t[:, :, 8:16]

                        # softplus(pre) = ln(exp(pre) + 1)
                        sp_t = opool.tile([128, 4, E], DT.float32, tag="sp")
                        nc.scalar.activation(sp_t[:], pre_ap, AFT.Exp)
                        nc.scalar.activation(sp_t[:], sp_t[:], AFT.Ln, bias=1.0)

                        nl_t = opool.tile([128, 4, E], DT.float32, tag="nl")
                        nc.vector.tensor_mul(nl_t[:], noise_t[:, 4 * h:4 * h + 4, :], sp_t[:])
                        nc.vector.tensor_add(nl_t[:], nl_t[:], gate_ap)

                        mx_t = opool.tile([128, 4, 8], DT.float32, tag="mx")
                        mi_t = opool.tile([128, 4, 8], DT.uint32, tag="mi")
                        for s in range(4):
                            nc.vector.max(mx_t[:, s, :], nl_t[:, s, :])
                            nc.vector.max_index(mi_t[:, s, :], mx_t[:, s, :], nl_t[:, s, :])

                        # p1 = 1/(1+e^(v2-v1)), p2 = e^(v2-v1)*p1
                        d_t = opool.tile([128, 4], DT.float32, tag="d")
                        nc.vector.tensor_sub(d_t[:], mx_t[:, :, 1], mx_t[:, :, 0])
                        e_t = opool.tile([128, 4], DT.float32, tag="e")
                        nc.scalar.activation(e_t[:], d_t[:], AFT.Exp)
                        pd_t = opool.tile([128, 4], DT.float32, tag="pd")
                        nc.vector.tensor_scalar_add(pd_t[:], e_t[:], 1.0)
                        p1_t = opool.tile([128, 4], DT.float32, tag="p1")
                        nc.vector.reciprocal(p1_t[:], pd_t[:])
                        p2_t = opool.tile([128, 4], DT.float32, tag="p2")
                        nc.vector.tensor_mul(p2_t[:], e_t[:], p1_t[:])

                        # scatter: r = (iota==idx1)*p1 + (iota==idx2)*p2
                        iota_b = iota_t[:].unsqueeze(1).broadcast_to((128, 4, 8))
                        r1_t = opool.tile([128, 4, 8], DT.float32, tag="r1")
                        nc.vector.tensor_tensor(
                            r1_t[:], iota_b, mi_t[:, :, 0:1].broadcast_to((128, 4, 8)),
                            ALU.is_equal)
                        nc.vector.tensor_mul(
                            r1_t[:], r1_t[:], p1_t[:].unsqueeze(2).broadcast_to((128, 4, 8)))
                        r2_t = opool.tile([128, 4, 8], DT.float32, tag="r2")
                        nc.vector.tensor_tensor(
                            r2_t[:], iota_b, mi_t[:, :, 1:2].broadcast_to((128, 4, 8)),
                            ALU.is_equal)
                        nc.vector.tensor_mul(
                            r2_t[:], r2_t[:], p2_t[:].unsqueeze(2).broadcast_to((128, 4, 8)))
                        nc.vector.tensor_add(r1_t[:], r1_t[:], r2_t[:])

                        nc.sync.dma_start(
                            rout_d[tok0:tok0 + 512, :].rearrange("(s p) e -> p s e", p=128),
                            r1_t[:])
                        nc.sync.dma_start(
                            texp_d[tok0:tok0 + 512, :].rearrange("(s p) e -> p s e", p=128),
                            mi_t[:, :, 0:2].bitcast(DT.int32))

            if loop_n > 1:
                with tc.For_i(0, loop_n, 1) as _i:
                    body(_i)
            else:
                body()

    nc.compile()
    return nc


def _pack_inputs(x, Wg, bg, Wn, bn, noise):
    xf = np.ascontiguousarray(np.asarray(x, dtype=np.float32).reshape(N_TOK, D))
    W = np.concatenate([np.asarray(Wg, np.float32), np.asarray(Wn, np.float32)], axis=0)
    Wh = W.astype(np.float16)
    Wl = ((W - Wh.astype(np.float32)) * SCALE).astype(np.float16)
    z16 = np.zeros_like(Wh.T)
    # rows 0:16 of the psum get Wh products; rows 32:48 (32-aligned for
    # PSUM partition-offset rules) get the 1/SCALE cross terms.
    whwl = np.ascontiguousarray(
        np.concatenate([Wh.T, z16, Wl.T], axis=1))                  # [D, 48]
    zwh = np.ascontiguousarray(
        np.concatenate([z16, z16, Wh.T], axis=1))                   # [D, 48]
    bcat = np.concatenate([np.asarray(bg, np.float32),
                           np.asarray(bn, np.float32)]).reshape(16, 1)
    ident = np.eye(16, dtype=np.float32)
    nf = np.asarray(noise, np.float32).reshape(N_TOK, E)

    in_maps = []
    for c in range(N_CORES):
        xs = xf[c * TC:(c + 1) * TC]
        xT = np.ascontiguousarray(xs.T)                              # [D, TC] f32
        xh = xT.astype(np.float16)
        xl = ((xT - xh.astype(np.float32)) * SCALE).astype(np.float16)
        in_maps.append(dict(
            xh=xh, xl=xl, whwl=whwl, zwh=zwh, bcat=bcat, ident=ident,
            noise=np.ascontiguousarray(nf[c * TC:(c + 1) * TC])))
    return in_maps


_PROGRAM_CACHE = {}


def kernel(x, Wg, bg, Wn, bn, noise, k):
    assert int(k) == 2
    in_maps = _pack_inputs(x, Wg, bg, Wn, bn, noise)
    key = (TC, 1024, 1)
    if key not in _PROGRAM_CACHE:
        _PROGRAM_CACHE[key] = build_program(TC, slab=1024, loop_n=1)
    nc = _PROGRAM_CACHE[key]
    res = run_bass_kernel_spmd(nc, in_maps, list(range(N_CORES)))
    rout = np.concatenate([res.results[c]["rout"] for c in range(N_CORES)], axis=0)
    texp = np.concatenate([res.results[c]["texp"] for c in range(N_CORES)], axis=0)
    return (rout.reshape(B, S, E).astype(np.float32),
            texp.reshape(B, S, 2).astype(np.int32))
